# revision 109
# baseline (speedup 1.0000x reference)
"""CTC loss kernel for Trainium2 (8 NeuronCores, data-parallel over batch).

Strategy (v3, PE-selector design: 57.14us; v2 was 72.9us, v1 122us)
------------------------------------------------------------------
B=128 samples, T=256, C=1024 classes, S=32 labels, E=2S+1=65 extended states.
Each of 8 cores handles 16 samples; host gathers and averages the loss.

Key idea vs v2: the host TRANSPOSES pred per sample-chunk to a
[128 class-partition, 8 block x 128 t] bf16 layout (classes permuted so the
sample's <=35 distinct label classes live in class-block 0).  Then:

 1. Act computes exp on mega-tiles [128, 4*1024] (4 sample-chunks per
    instruction, bf16 in/out, NO accum_out) -- 3598ns per mega vs v2's
    4*1225ns: the softmax row-sums move off Act entirely.  Mega 0 is split
    in two + fed by half-DMAs on both queues so exp 0 starts at ~1.7us.
    Act cost is free-size-only (no dtype speedup), so bf16 buys DMA, not
    exp; accum_out removal and instruction batching buy the exp time.
 2. PE (otherwise idle; ldweights is FREE in the cost model and matmult
    costs out-free-size x pe_cycle) does per sample-chunk:
      - q extraction: qz[128 t, 65] = et_block0^T @ Msel_s, Msel a
        per-sample selector matrix with C_SEL = bf16(0.53) at
        [slot_col[s, e], e] for live states, zero columns for dead states.
        Replaces v2's Pool indirect_copy + DVE qmul (~100ns/sample-chunk).
      - Z_t sums: 8 accumulating ones-matmuls -> pz[th][:, s] (1ns each).
 3. q is UNNORMALIZED: q = C_SEL*exp(x) (no divide by Z_t; fp8 range
    [0.003, 90]).  The per-t softmax normalizer is recovered exactly on
    the host via lnzsum = sum_t ln Z_t, computed on-device: Act Ln on the
    pz [128, 16] PSUM columns (198ns/chunk) + a free PE partition-sum
    matmul into zsum cols.  Alpha magnitude drift matches v2 (the ratio
    of the two schemes' q is Z_t/e^7.43 = e^{+-0.04}), but the PER-SAMPLE
    ll spread (+-60 nats over full T) still requires the single t=127
    renorm by Z_b (without it bf16 sel underflows).
 4. fp8 ring conversion = batched copies from the shared 4-bank PSUM tile
    (GPSIMD cannot touch PSUM per birverifier): ch0 megas on DVE (idle
    before DP0), ch1 as two Act half-copies pinned after the last exp.
 5. DRAM bounce (q_ring [t-part] -> qd[s][th][t][e] -> qh [s-part]) is the
    only partition-regroup mechanism (DMA cost = per-PARTITION-bytes *
    0.3855ns, so reloads are t-split across queues; a reload behind its
    stores on the SAME queue skips the +900ns cross-DMA sem).
 6. CTC DP on DVE as v2: per (state, chunk) fused tensor_tensor_scan
    alpha_t = q_t * (alpha_{t-1} + u_t); odd-state u via 2x-mode bf16
    tensor_tensor where every sample's adjacent labels differ (allm1),
    scalar_tensor_tensor otherwise.  Chunk-0 scans start at t0 = e//2
    (state e unreachable earlier); the skipped triangle is pre-zeroed by
    one strided memset in the pre-DP0 idle window.  Scans have NO DVE perf
    modes (194ns at TCH=128 regardless of dtype).
 7. Device returns (sel, Z_b, lnzsum) per sample; host:
    ll = ln(sel) + ln(Z_b) - T*ln(C_SEL) - lnzsum, loss = mean(-ll/len).
    Result staging (zb copy, zsum PSUM escape, zsum0+zsum1 via
    Relu-with-AP-bias) runs on the idle Act engine, off the DVE tail.

Schedule (CoreSim cost model, per core): stream exp gapless 1.7->30.7us;
DP0 starts ~21.8 (ch0 exps end 16.3 + copy/store/reload/sem chain ~5.5);
DVE chain (DP0+renorm+DP1) runs gapless to ~54.5; res DMA + final barrier
~2.9.  DP0-end (not qh1-readiness, ~36.5) gates DP1, so the stream is not
the binding path.  Engine busy: DVE ~33us, Act ~31us, SP/Pool ~19us,
PE ~1us.

Toolchain notes: walrus accepts at most ONE sync wait per instruction
(_legalize_waits splits extras onto single-wait NoOps), rejects
TensorScalarPtr AND tensor_tensor_scan on Pool, rejects GPSIMD PSUM
access, and fails codegen on InstTensorTensorReduce ("ISA wrong length").
DMA cannot read PSUM or cast (except gpsimd); matmul operands must be
SBUF, out PSUM fp32 (bf16 operands: 1 cycle/row, fp32: 4).  The
TileScheduler fixes each engine's instruction ORDER from its own
predicted ready times -- a late-ready instruction placed early
head-of-line blocks the engine (hence the add_dep_helper pins).
"""

import numpy as np
from ml_dtypes import bfloat16 as np_bf16

B, T, C, S = 128, 256, 1024, 32
E = 2 * S + 1            # 65
NCORES = 8
BPC = B // NCORES        # 16 samples per core
TCH = 128                # T-chunk length (renorm folded at the boundary)
NIDX = 80                # ring slot stride (65 used)
NBLK = C // 128          # 8 class blocks per sample-chunk
MEGA = 4                 # sample-chunks per Act exp instruction

C_SEL = float(np_bf16(0.53))   # per-step scale folded into Msel (bf16-exact)
LN_C = float(np.log(C_SEL))

_compiled = None


def _build_host_tensors(pred, target, length):
    """Slice/derive per-core input tensors (host-side marshalling only).

    predT[s, th] is the [128 class-partition, 8*128 t] bf16 transpose of the
    sample's chunk-th logits, classes permuted so the sample's distinct
    label classes (blank + up to 32 labels) occupy classes [0, 35) (softmax
    is permutation-invariant).  Msel[s] is the [128, 65] selector matrix
    with C_SEL at [slot_col[s, e], e] for live states e, zero columns for
    dead states.
    """
    pred = np.ascontiguousarray(np.asarray(pred, dtype=np.float32))
    target = np.asarray(target).astype(np.int64)
    length = np.asarray(length).astype(np.int64)

    in_maps = []
    for c in range(NCORES):
        sl = slice(c * BPC, (c + 1) * BPC)
        tg = target[sl]          # [16, 32]
        ln = length[sl]          # [16]

        predT = np.empty((BPC, 2, 128, NBLK * TCH), dtype=np_bf16)
        msel = np.zeros((128, BPC * E), dtype=np_bf16)
        for s in range(BPC):
            classes = [0]        # blank first
            seen = {0: 0}
            for k in range(S):
                v = int(tg[s, k])
                if v not in seen:
                    seen[v] = len(classes)
                    classes.append(v)
            rest = np.setdiff1d(np.arange(C), np.array(classes))
            perm = np.concatenate([np.array(classes), rest])
            ps = pred[c * BPC + s][:, perm]          # [T, C] permuted classes
            # predT[s, th][p, b*128+t] = ps[th*128+t, b*128+p]
            pst = ps.reshape(2, TCH, NBLK, 128)      # [th, t, b, p]
            predT[s] = pst.transpose(0, 3, 2, 1).reshape(2, 128, NBLK * TCH)
            for e in range(E):
                if e > 2 * ln[s]:
                    continue                         # dead state: zero col
                v = 0 if e % 2 == 0 else int(tg[s, (e - 1) // 2])
                msel[seen[v], s * E + e] = C_SEL

        # skip mask m[s, e] (odd e >= 3): label differs from previous label
        msb = np.zeros((BPC, E), dtype=np.float32)
        for s in range(BPC):
            for k in range(1, S):
                e = 2 * k + 1
                msb[s, e] = 1.0 if tg[s, k] != tg[s, k - 1] else 0.0

        # final-state selector: states 2L and 2L-1
        emask = np.zeros((BPC, E), dtype=np.float32)
        emask[np.arange(BPC), 2 * ln] = 1.0
        emask[np.arange(BPC), 2 * ln - 1] = 1.0

        in_maps.append(
            {
                "predT": predT.reshape(BPC * 2 * 128, NBLK * TCH),
                "msel": msel,
                "msb": msb,
                "emask": emask,
            }
        )
    return in_maps, length


def _build_program(allm1=frozenset()):
    """allm1: odd states e where EVERY sample in the batch has skip-mask 1
    (adjacent labels differ): u = alpha[e-1] + alpha[e-2] is a plain
    tensor_tensor, which gets the DVE 2x bf16 mode (127ns vs 194ns)."""
    import concourse.bass as bass
    import concourse.tile as tile
    from concourse import mybir
    from concourse.tile import add_dep_helper

    f32 = mybir.dt.float32
    bf16 = mybir.dt.bfloat16
    f8 = mybir.dt.float8e4
    AF = mybir.ActivationFunctionType
    OP = mybir.AluOpType

    nc = bass.Bass()
    predT = nc.declare_dram_parameter(
        "predT", [BPC * 2 * 128, NBLK * TCH], bf16, isOutput=False
    )
    msel = nc.declare_dram_parameter("msel", [128, BPC * E], bf16, isOutput=False)
    msb = nc.declare_dram_parameter("msb", [BPC, E], f32, isOutput=False)
    emask = nc.declare_dram_parameter("emask", [BPC, E], f32, isOutput=False)
    res = nc.declare_dram_parameter("res", [BPC, 3], f32, isOutput=True)

    with tile.TileContext(nc) as tc:
        with (
            tc.tile_pool(name="persist", bufs=1) as pp,
            tc.tile_pool(name="et_p", bufs=3) as et_p,
            tc.tile_pool(name="psq", bufs=1, space="PSUM") as psq,
            tc.tile_pool(name="psz", bufs=1, space="PSUM") as psz,
            tc.tile_pool(name="dram", bufs=1, space="DRAM") as dram_p,
        ):
            # persistent tensors
            msel_sb = pp.tile([128, BPC * E], bf16, tag="msel_sb")
            m_sb = pp.tile([BPC, E], f32, tag="m_sb")
            emask_sb = pp.tile([BPC, E], f32, tag="emask_sb")
            ones_bf = pp.tile([128, 1], bf16, tag="ones_bf")
            ones_f32 = pp.tile([128, 1], f32, tag="ones_f32")
            lnz_sb = [
                pp.tile([128, BPC], f32, tag=f"lnz{th}", name=f"lnz{th}")
                for th in range(2)
            ]
            zsum_sb = pp.tile([BPC, 2], f32, tag="zsum_sb")
            # q ring: one [128 t, NIDX] block per sample-chunk (32 slots)
            q_ring = pp.tile([128, 32 * NIDX], f8, tag="q_ring")
            # DRAM bounce: per sample, per chunk, [t][e] (e contiguous)
            qd = dram_p.tile([BPC, 2 * TCH * E], f8, tag="qd")
            qh = [
                pp.tile([BPC, TCH, E], f8, tag="qh0", name="qh0"),
                pp.tile([BPC, TCH, E], f8, tag="qh1", name="qh1"),
            ]
            # alpha buffer: row 0 = zero state, col 0 = t=-1 zeros;
            # A[:, e+1, 1+t] = alpha[e, t]
            alpha = pp.tile([BPC, E + 1, T + 1], bf16, tag="alpha")
            ubuf = pp.tile([BPC, TCH], bf16, tag="ubuf")
            zb_t = pp.tile([BPC, 1], f32, tag="zb")
            rb_t = pp.tile([BPC, 1], f32, tag="rb")
            resbuf = pp.tile([BPC, 3], f32, tag="resbuf")
            selbuf = pp.tile([BPC, E], f32, tag="selbuf")
            # PSUM: per-chunk Z_t columns + 2-col zsum, packed in one bank
            pzall = psz.tile([128, 2 * BPC + 2], f32, tag="pzall")
            # q matmul outputs: one persistent 4-bank tile; mega m uses the
            # 512-f32 slot m%4 (sample j of the mega at cols slot*512+j*128)
            qzt = psq.tile([128, MEGA * 512], f32, tag="qzt")

            def pz_col(th, s):
                return pzall[:, th * BPC + s : th * BPC + s + 1]

            def pz_full(th):
                return pzall[:, th * BPC : (th + 1) * BPC]

            def zsum_col(th):
                return pzall[0:BPC, 2 * BPC + th : 2 * BPC + th + 1]

            # warm the Act exp table before the first tile lands
            warm = pp.tile([128, 1], f32, tag="warm")
            nc.vector.memset(warm[:], 0.0)
            nc.scalar.activation(warm[:], warm[:], AF.Exp)
            nc.vector.memset(ones_bf[:], 1.0)
            nc.vector.memset(ones_f32[:], 1.0)
            # zero row 0 (both chunks) and column 0 of the alpha buffer
            nc.vector.memset(alpha[:, 0, :], 0.0)
            nc.vector.memset(alpha[:, :, 0:1].rearrange("p e one -> p (e one)"), 0.0)
            # forward-triangle: state e is unreachable for t < e//2, so
            # chunk-0 scans start at t0=e//2; pre-zero the skipped cells
            # (cols 1..32 of rows 3..65) so u-ops read exact zeros.  Runs
            # during the pre-DP0 DVE idle window.
            nc.vector.memset(alpha[:, 3 : E + 1, 1:33], 0.0)


            def pred_queue(sc):
                return nc.sync if sc % 2 == 0 else nc.gpsimd

            def emit_store(th, quad, quads=1, eng=None):
                # batched store: ring slots (samples 4q.., chunk th) ->
                # qd[s][th][t][e]; DRAM AP leads with t, ends with the
                # contiguous e dim.
                s0 = 4 * quad
                ns = 4 * quads
                dst = (
                    qd[s0 : s0 + ns, th * TCH * E : (th + 1) * TCH * E]
                    .rearrange("s (t e) -> t s e", t=TCH)
                )
                r0 = BPC * th + s0
                src = (
                    q_ring[:, :]
                    .rearrange("p (s i) -> p s i", i=NIDX)
                    [:, r0 : r0 + ns, 0:E]
                )
                return (eng or nc.sync).dma_start(out=dst, in_=src)

            def emit_reload(th, t0, t1, eng):
                # t-range reload: DMA cost is per-partition-bytes, so
                # t-splitting divides the transfer time across queues
                eng.dma_start(
                    out=qh[th][:, t0:t1, :].rearrange("p t e -> p (t e)"),
                    in_=qd[:, th * TCH * E + t0 * E : th * TCH * E + t1 * E],
                )

            # ---- stream phase: 8 megas of 4 sample-chunks ----
            for m in range(8):
                th = m // 4
                et = et_p.tile([128, MEGA * NBLK * TCH], bf16, tag="et")
                for j in range(MEGA):
                    sc = MEGA * m + j
                    s = sc % BPC
                    row = (s * 2 + th) * 128
                    if m == 0:
                        # half-DMAs on both queues: cut first-exp latency
                        nc.sync.dma_start(
                            out=et[:, j * C : j * C + C // 2],
                            in_=predT[row : row + 128, 0 : C // 2],
                        )
                        nc.gpsimd.dma_start(
                            out=et[:, j * C + C // 2 : (j + 1) * C],
                            in_=predT[row : row + 128, C // 2 : C],
                        )
                    else:
                        pred_queue(sc).dma_start(
                            out=et[:, j * C : (j + 1) * C],
                            in_=predT[row : row + 128, :],
                        )
                if m == 0:
                    # msel needed by the first PE matmuls (~8us); emitted
                    # after mega-0's half-DMAs so it doesn't delay exp 0,
                    # but before the matmuls so the dep is tracked
                    nc.gpsimd.dma_start(out=msel_sb[:], in_=msel[:])
                # exp in place; mega 0 as two halves (earlier first exp)
                if m == 0:
                    nc.scalar.activation(
                        et[:, 0 : 2 * C], et[:, 0 : 2 * C], AF.Exp
                    )
                    last_exp = nc.scalar.activation(
                        et[:, 2 * C : 4 * C], et[:, 2 * C : 4 * C], AF.Exp
                    )
                else:
                    last_exp = nc.scalar.activation(et[:], et[:], AF.Exp)
                slot = (m % 4) * 512
                for j in range(MEGA):
                    sc = MEGA * m + j
                    s = sc % BPC
                    # q extraction: out[128 t, 65] = et_block0^T @ Msel_s
                    nc.tensor.matmul(
                        qzt[:, slot + j * 128 : slot + j * 128 + E],
                        lhsT=et[:, j * C : j * C + 128],
                        rhs=msel_sb[:, s * E : (s + 1) * E],
                        start=True,
                        stop=True,
                    )
                    # Z_t: 8 accumulating ones-matmuls -> pz[th][:, s]
                    for b in range(NBLK):
                        nc.tensor.matmul(
                            pz_col(th, s),
                            lhsT=et[:, j * C + b * 128 : j * C + (b + 1) * 128],
                            rhs=ones_bf[:],
                            start=(b == 0),
                            stop=(b == NBLK - 1),
                        )
                if th == 0:
                    # ch0 fp8 ring conversion: batched DVE copies (idle
                    # before DP0; GPSIMD cannot access PSUM), then the quad
                    # store.  Mega 3 is copied in two halves so the critical
                    # quad-3 store isn't gated on the whole-mega copy.
                    sc0 = MEGA * m
                    nc.vector.tensor_copy(
                        out=q_ring[:, :]
                        .rearrange("p (s i) -> p s i", i=NIDX)
                        [:, sc0 : sc0 + MEGA, 0:E],
                        in_=qzt[:, slot : slot + 512]
                        .rearrange("p (j e) -> p j e", e=128)[:, :, 0:E],
                    )
                    emit_store(0, m % 4)
                if m == 1:
                    nc.sync.dma_start(out=m_sb[:], in_=msb[:])
                    nc.sync.dma_start(out=emask_sb[:], in_=emask[:])
                if m == 3:
                    # ch0 reload: SP piece rides behind the SP stores (no
                    # cross-DMA sem), so it can be the bigger one; the Pool
                    # piece pays the +900ns store sem, so keep it smaller
                    emit_reload(0, 0, 58, nc.sync)
                    emit_reload(0, 58, TCH, nc.gpsimd)

            # ch1 fp8 conversion: two Act half-copies right after the last
            # exp; each half's store+reload chains on ONE queue (SP / Pool)
            # so no cross-DMA sems are paid.
            for half, eng in ((0, nc.sync), (1, nc.gpsimd)):
                h = nc.scalar.activation(
                    q_ring[:, :]
                    .rearrange("p (s i) -> p s i", i=NIDX)
                    [:, BPC + 8 * half : BPC + 8 * half + 8, 0:E],
                    qzt[:, :]
                    .rearrange("p (s e) -> p s e", e=128)
                    [:, 8 * half : 8 * half + 8, 0:E],
                    AF.Copy,
                )
                add_dep_helper(
                    h.ins, last_exp.ins,
                    reason="ch1 fp8 copy after the exp stream",
                )
                last_ch1_copy = h
                emit_store(1, 2 * half, quads=2, eng=eng)
            emit_reload(1, 0, 43, nc.sync)
            emit_reload(1, 43, 86, nc.gpsimd)
            emit_reload(1, 86, TCH, nc.scalar)

            # Z -> ln -> partition-sum (off the critical path; pinned after
            # the ch1 copies so the Act queue finishes the qh1 chain first)
            for th in range(2):
                hl = nc.scalar.activation(lnz_sb[th][:], pz_full(th), AF.Ln)
                add_dep_helper(
                    hl.ins, last_ch1_copy.ins,
                    reason="lnz after the ch1 fp8 copies",
                )
                nc.tensor.matmul(
                    zsum_col(th), lhsT=lnz_sb[th][:, 0:BPC],
                    rhs=ones_f32[:], start=True, stop=True,
                )

            def dp_chunk(th):
                lo = th * TCH          # alpha-buffer col for t = th*128 - 1
                last = None
                for e in range(E):
                    t0 = e // 2 if th == 0 else 0   # forward triangle
                    if e >= 3 and e % 2 == 1:
                        # u = alpha[e-2]*m + alpha[e-1]  (buffer rows e-1, e)
                        if e in allm1:
                            nc.vector.tensor_tensor(
                                out=ubuf[:, 0 : TCH - t0],
                                in0=alpha[:, e - 1, lo + t0 : lo + TCH],
                                in1=alpha[:, e, lo + t0 : lo + TCH],
                                op=OP.add,
                            )
                        else:
                            nc.vector.scalar_tensor_tensor(
                                ubuf[:, 0 : TCH - t0],
                                alpha[:, e - 1, lo + t0 : lo + TCH],
                                m_sb[:, e : e + 1],
                                alpha[:, e, lo + t0 : lo + TCH],
                                OP.mult,
                                OP.add,
                            )
                        u_ap = ubuf[:, 0 : TCH - t0]
                    else:
                        u_ap = alpha[:, e, lo + t0 : lo + TCH]
                    if th == 0:
                        init = 1.0 if e <= 1 else 0.0
                    else:
                        init = alpha[:, e + 1, lo : lo + 1]
                    # alpha_t = q_t * (alpha_{t-1} + u_t)
                    last = nc.vector.tensor_tensor_scan(
                        out=alpha[:, e + 1, lo + 1 + t0 : lo + 1 + TCH],
                        data0=u_ap,
                        data1=qh[th][:, t0:TCH, e],
                        initial=init,
                        op0=OP.add,
                        op1=OP.mult,
                    )
                return last

            dp_chunk(0)

            # boundary renorm at t=127: the PER-SAMPLE ll spread (+-60
            # nats over the full T) would underflow bf16 without it
            nc.vector.tensor_reduce(
                out=zb_t[:],
                in_=alpha[:, 1 : E + 1, TCH : TCH + 1],
                op=OP.add,
                axis=mybir.AxisListType.XY,
            )
            nc.vector.reciprocal(rb_t[:], zb_t[:])
            nc.vector.tensor_scalar(
                alpha[:, :, TCH : TCH + 1].rearrange("p e one -> p (e one)"),
                alpha[:, :, TCH : TCH + 1].rearrange("p e one -> p (e one)"),
                rb_t[:],
                None,
                OP.mult,
            )

            last_scan = dp_chunk(1)

            # final: sel = sum_e emask * alpha[., e, 255]
            # (tensor_tensor_reduce would fuse these but fails walrus
            # codegen: "ISA wrong length")
            nc.vector.tensor_tensor(
                out=selbuf[:],
                in0=alpha[:, 1 : E + 1, T : T + 1].rearrange("p e one -> p (e one)"),
                in1=emask_sb[:],
                op=OP.mult,
            )
            nc.vector.tensor_reduce(
                out=resbuf[:, 0:1], in_=selbuf[:], op=OP.add,
                axis=mybir.AxisListType.X,
            )
            # result staging on Act (idle after its stream; keeps these ops
            # off the DVE chain/tail): zb copy, zsum PSUM->SBUF copy, and
            # lnzsum = zsum0 + zsum1 as Relu(zsum0 + bias) -- zsum ~ +1900
            # so Relu is the identity, and non-Copy funcs accept an AP bias
            # (GPSIMD cannot read PSUM; Copy rejects AP bias).
            nc.scalar.activation(resbuf[:, 1:2], zb_t[:], AF.Copy)
            nc.scalar.activation(
                zsum_sb[:], pzall[0:BPC, 2 * BPC : 2 * BPC + 2], AF.Copy
            )
            nc.scalar.activation(
                resbuf[:, 2:3], zsum_sb[:, 0:1], AF.Relu,
                bias=zsum_sb[:, 1:2],
            )
            nc.sync.dma_start(out=res[:], in_=resbuf[:])

    return nc


def _legalize_waits(nc):
    """This toolchain's walrus accepts at most ONE sync-wait (and one update)
    per instruction (the 64B Events field).  Tile emits multi-wait
    instructions; split the extras onto single-wait NoOps placed just before
    (waits) / after (updates, non-DMA only) on the same engine."""
    from concourse import mybir

    for fn in nc.m.functions:
        for bb in fn.blocks:
            out = []
            for inst in bb.instructions:
                si = inst.sync_info
                if si is None:
                    out.append(inst)
                    continue
                waits = list(si.on_wait or [])
                updates = list(si.on_update or [])
                for w in waits[:-1]:
                    out.append(
                        mybir.InstNoOp(
                            name=f"{inst.name}_w{len(out)}",
                            ins=[],
                            outs=[],
                            engine=inst.engine,
                            sync_info=mybir.SyncInfo(on_wait=[w], on_update=[]),
                        )
                    )
                post = []
                if len(updates) > 1:
                    is_dma = "DMA" in type(inst).__name__
                    assert not is_dma, f"DMA with multiple updates: {inst.name}"
                    for u in updates[1:]:
                        post.append(
                            mybir.InstNoOp(
                                name=f"{inst.name}_u{len(post)}",
                                ins=[],
                                outs=[],
                                engine=inst.engine,
                                sync_info=mybir.SyncInfo(on_wait=[], on_update=[u]),
                            )
                        )
                    updates = updates[:1]
                inst.sync_info = mybir.SyncInfo(
                    on_wait=waits[-1:], on_update=updates
                )
                out.append(inst)
                out.extend(post)
            bb.instructions = out


def _allm1_states(target):
    """Odd states e=2k+1 where every sample's labels k-1, k differ."""
    target = np.asarray(target)
    diff = target[:, 1:] != target[:, :-1]          # [B, S-1]
    return frozenset(
        2 * k + 1 for k in range(1, S) if bool(diff[:, k - 1].all())
    )


def _get_program(allm1=frozenset()):
    global _compiled
    if _compiled is None:
        _compiled = _build_program(allm1)
        _legalize_waits(_compiled)  # hw/walrus only; CoreSim needs the raw form
    return _compiled


def kernel(pred, target, length, batch_size):
    from concourse.bass_utils import run_bass_kernel_spmd

    in_maps, length_np = _build_host_tensors(pred, target, length)
    nc = _get_program(_allm1_states(target))
    out = run_bass_kernel_spmd(nc, in_maps, list(range(NCORES)))

    sel = np.concatenate([r["res"][:, 0] for r in out.results])
    zb = np.concatenate([r["res"][:, 1] for r in out.results])
    lnzsum = np.concatenate([r["res"][:, 2] for r in out.results])
    ll = np.log(sel) + np.log(zb) - np.float32(T * LN_C) - lnzsum
    loss = np.mean(-(ll / length_np.astype(np.float32)))
    return np.float32(loss)
